# revision 23
# baseline (speedup 1.0000x reference)
"""Masked 16-bin histogram (BinsCount) on 8 TRN2 NeuronCores.

Reference computation (see problem):
    m = broadcast(mask != 0); in_range = (x >= e0) & (x <= e16)
    idx = clip(searchsorted(edges, x, 'right') - 1, 0, 15)
    hist = bincount(idx, weights=m & in_range); density = hist / sum(m)
    returns (x, density)

Device algorithm: for j in 0..16: S_j = #{x' >= e_j}, plus T = #{x' > e16},
where x' = fp16(x) + m'' and m'' = 0 where mask kept, -inf where dropped
(so dropped elements become -inf and vanish from every count).  Exactly:
    hist[k] = S_k - S_{k+1}  (k < 15),   hist[15] = S_15 - T
n_selected = 32 * count(mask != 0) (host, exact).

Engine split per [128, 2048] tile (cost-model rates):
    gpsimd SWDGE DMA : cast-load x f32->fp16, then SBUF->SBUF accum-add of
                       the resident fp16 mask tile (masking costs no
                       compute-engine time)          ~3.2 us
    VectorE          : 14 edges, fused is_ge+accum fp16 (~0.61 us each)
    ScalarE          : 4 edges via Sign activation + accum (~2.1 us each)

Sharding: data-parallel along S (dim 2): core i takes x[:, :, i*256:(i+1)*256, :]
(64 MiB) + matching mask rows; the (B,H)-broadcast of the mask is handled
on-device by tile layout alignment.

fp16 note: x is rounded to fp16 before binning; only elements within half
an fp16 ulp of a bin edge can change bins (~0.1% of a bin's mass), far
inside the 2e-2 tolerance.  Counts themselves are exact integers in f32
accumulators.
"""

import os
import sys

import numpy as np

for _p in ("/opt/trn_rl_repo",):
    if _p not in sys.path:
        sys.path.insert(0, _p)

B, H, S, C = 2, 16, 2048, 2048
NCORES = 8
S_SH = S // NCORES              # 256 rows of S per core
TILES = (B * H * S_SH) // 128   # 64 tiles of [128, C] per core
NEDGE = 18                      # S_0..S_16 (>=), T (> e16)

# Edge -> engine assignment (indices into the 18 logical counters).
DVE_EDGES = tuple(range(0, 6))       # fused ts+accum on VectorE
DMA_EDGES = tuple(range(6, 12))      # 4x compare on VectorE + SWDGE accum-add
ACT_EDGES = tuple(range(12, 18))     # Sign+accum on ScalarE

LAST_EXEC_NS = None
LAST_RESULTS = None


def _build(edges17):
    """Raw-bass SPMD graph; edges17: 17 python floats (sorted)."""
    import concourse.bass as bass
    import concourse.mybir as mybir

    f32 = mybir.dt.float32
    f16 = mybir.dt.float16
    Alu = mybir.AluOpType
    ACT = mybir.ActivationFunctionType

    MBUF = 8                 # xm ring (gps DMA -> DVE/ACT readers)
    IRING = 2                # indicator ring depth (DVE -> SWDGE accum)
    ND = len(DMA_EDGES)
    RS = S_SH // 128         # mask row-tiles (2)

    def cmp_of(e):
        return Alu.is_ge if e < 17 else Alu.is_gt

    def val_of(e):
        return float(edges17[min(e, 16)])

    nc = bass.Bass()
    x_ext = nc.declare_dram_parameter("x", [B * H, RS, 128, C], f32,
                                      isOutput=False)
    mp_ext = nc.declare_dram_parameter("mp", [RS, 128, C], f16,
                                       isOutput=False)
    bias_ext = nc.declare_dram_parameter("bias", [128, NEDGE], f32,
                                         isOutput=False)
    out_ext = nc.declare_dram_parameter("out", [128, NEDGE], f32, isOutput=True)

    with (
        nc.sbuf_tensor([128, MBUF * C], f16) as xmbuf,
        nc.sbuf_tensor([128, RS * C], f16) as mbuf,
        nc.sbuf_tensor([128, NEDGE], f32) as biasbuf,
        nc.sbuf_tensor([128, C], f16) as scr_v,
        nc.sbuf_tensor([128, C], f16) as scr_a,
        nc.sbuf_tensor([128, max(1, IRING * ND) * C], f16) as indbuf,
        nc.sbuf_tensor([128, max(1, ND) * C], f16) as accd,
        nc.sbuf_tensor([128, NEDGE * TILES], f32) as acc,
        nc.sbuf_tensor([128, NEDGE], f32) as accf,
        nc.semaphore("dma_b") as dma_b,     # sync loads: bias, mask
        nc.semaphore("xc_sem") as xc_sem,   # gps cast-load completions
        nc.semaphore("xm_sem") as xm_sem,   # gps mask-add completions
        nc.semaphore("ind_sem") as ind_sem,  # DVE indicator (t,j) -> 6t+j+1
        nc.semaphore("dma_acc") as dma_acc,  # SWDGE accum completions (16 ea)
        nc.semaphore("zinit") as zinit,     # accd zero-init done
        nc.semaphore("vdone") as vdone,     # DVE finished tile t -> t+1
        nc.semaphore("adone") as adone,     # ACT finished tile t -> t+1
        nc.semaphore("done_sem") as done_sem,
        nc.Block() as block,
    ):
        def slot_ap(e, t):
            return acc[:, e * TILES + t: e * TILES + t + 1]

        @block.sync
        def _(sync):
            sync.dma_start(out=biasbuf[:], in_=bias_ext[:]).then_inc(dma_b, 16)
            for r in range(RS):
                sync.dma_start(out=mbuf[:, r * C:(r + 1) * C],
                               in_=mp_ext[r]).then_inc(dma_b, 16)
            sync.wait_ge(done_sem, 1)
            sync.dma_start(out=out_ext[:], in_=accf[:]).then_inc(dma_b, 16)

        @block.gpsimd
        def _(gpsimd):
            gpsimd.wait_ge(dma_b, 16 * (1 + RS))
            if ND:
                gpsimd.wait_ge(zinit, 1)
            for t in range(TILES):
                bh, r = divmod(t, RS)
                m = t % MBUF
                mslot = xmbuf[:, m * C:(m + 1) * C]
                if t >= MBUF:
                    # xm slot reused: DVE/ACT readers of tile t-MBUF done
                    gpsimd.wait_ge(vdone, t - MBUF + 1)
                    gpsimd.wait_ge(adone, t - MBUF + 1)
                # cast-load f32 -> fp16
                gpsimd.dma_start(out=mslot,
                                 in_=x_ext[bh, r]).then_inc(xc_sem, 16)
                gpsimd.wait_ge(xc_sem, 16 * (t + 1))
                # fp16 masking: += (0 | -inf)
                gpsimd.dma_start(out=mslot, in_=mbuf[:, r * C:(r + 1) * C],
                                 accum_op=Alu.add).then_inc(xm_sem, 16)
                # SWDGE-accumulated edges: accd[j] += ind16(t, j).
                # Full serialization of accum transfers (wait for ALL prior
                # accum completions before enqueueing) keeps RMW race-free
                # even if descriptors fan out across queues.
                base = (t % IRING) * ND
                for j in range(ND):
                    gpsimd.wait_ge(ind_sem, ND * t + j + 1)
                    if t > 0 or j > 0:
                        gpsimd.wait_ge(dma_acc, 16 * (ND * t + j))
                    gpsimd.dma_start(
                        out=accd[:, j * C:(j + 1) * C],
                        in_=indbuf[:, (base + j) * C:(base + j + 1) * C],
                        accum_op=Alu.add).then_inc(dma_acc, 16)

        @block.vector
        def _(vector):
            # zero-init the SWDGE accumulators before any accum-add lands
            if ND:
                nc.vector.memset(accd[:], 0.0).then_inc(zinit, 1)
            for t in range(TILES):
                m = t % MBUF
                xs = xmbuf[:, m * C:(m + 1) * C]
                vector.wait_ge(xm_sem, 16 * (t + 1))
                if ND and t >= IRING:
                    # indicator ring slot reuse: all accums of tile t-IRING
                    # must have completed
                    vector.wait_ge(dma_acc, 16 * ND * (t - IRING + 1))
                base = (t % IRING) * ND
                for j, e in enumerate(DMA_EDGES):
                    nc.vector.tensor_scalar(
                        out=indbuf[:, (base + j) * C:(base + j + 1) * C],
                        in0=xs, scalar1=val_of(e), scalar2=None,
                        op0=cmp_of(e)).then_inc(ind_sem, 1)
                last = None
                for e in DVE_EDGES:
                    last = nc.vector.tensor_scalar(
                        out=scr_v[:], in0=xs,
                        scalar1=val_of(e), scalar2=None,
                        op0=cmp_of(e), op1=Alu.add, accum_out=slot_ap(e, t))
                last.then_inc(vdone, 1)
            vector.wait_ge(adone, TILES)
            if ND:
                vector.wait_ge(dma_acc, 16 * ND * TILES)
            for e in range(NEDGE):
                if e in DMA_EDGES:
                    j = DMA_EDGES.index(e)
                    ins = nc.vector.tensor_reduce(
                        out=accf[:, e:e + 1],
                        in_=accd[:, j * C:(j + 1) * C],
                        axis=mybir.AxisListType.X, op=Alu.add)
                else:
                    ins = nc.vector.tensor_reduce(
                        out=accf[:, e:e + 1],
                        in_=acc[:, e * TILES:(e + 1) * TILES],
                        axis=mybir.AxisListType.X, op=Alu.add)
            ins.then_inc(done_sem, 1)

        @block.scalar
        def _(scalar):
            scalar.wait_ge(dma_b, 16)  # bias loaded
            for t in range(TILES):
                m = t % MBUF
                scalar.wait_ge(xm_sem, 16 * (t + 1))
                last = None
                for e in ACT_EDGES:
                    last = nc.scalar.activation(
                        out=scr_a[:], in_=xmbuf[:, m * C:(m + 1) * C],
                        func=ACT.Sign, bias=biasbuf[:, e:e + 1], scale=1.0,
                        accum_out=slot_ap(e, t))
                last.then_inc(adone, 1)

    return nc


_BUILD_CACHE = {}


def _get_nc(edges17):
    key = (tuple(edges17), DVE_EDGES, ACT_EDGES)
    if key not in _BUILD_CACHE:
        _BUILD_CACHE[key] = _build(edges17)
    return _BUILD_CACHE[key]


def kernel(x, mask, bins_edges):
    global LAST_EXEC_NS, LAST_RESULTS
    from concourse.bass_utils import run_bass_kernel_spmd

    x = np.asarray(x)
    mask = np.asarray(mask)
    edges = np.asarray(bins_edges, dtype=np.float32)
    assert x.shape == (B, H, S, C) and edges.shape == (17,)

    m2 = (mask.reshape(S, C) != 0)
    n_sel = np.float32(B * H) * np.float32(m2.sum(dtype=np.int64))
    mp16 = np.where(m2, np.float16(0.0), np.float16(-np.inf)).astype(np.float16)

    nc = _get_nc([float(v) for v in edges])

    bias = np.tile(-edges[np.minimum(np.arange(NEDGE), 16)].reshape(1, -1),
                   (128, 1)).astype(np.float32)
    in_maps = []
    for i in range(NCORES):
        xs = np.ascontiguousarray(
            x[:, :, i * S_SH:(i + 1) * S_SH, :]
        ).reshape(B * H, S_SH // 128, 128, C)
        mps = mp16[i * S_SH:(i + 1) * S_SH].reshape(S_SH // 128, 128, C)
        in_maps.append({"x": xs, "mp": np.ascontiguousarray(mps),
                        "bias": bias})

    trace = bool(int(os.environ.get("HIST_TRACE", "0")))
    res = run_bass_kernel_spmd(nc, in_maps, list(range(NCORES)), trace=trace)
    LAST_RESULTS = res
    LAST_EXEC_NS = res.exec_time_ns

    S_tot = np.zeros(NEDGE, dtype=np.float64)
    for i in range(NCORES):
        S_tot += res.results[i]["out"].astype(np.float64).sum(axis=0)

    n_elem_total = float(B * H * S * C)
    for e in ACT_EDGES:
        S_tot[e] = (S_tot[e] + n_elem_total) / 2.0

    hist = np.empty(16, dtype=np.float64)
    hist[:15] = S_tot[:15] - S_tot[1:16]
    hist[15] = S_tot[15] - S_tot[17]

    density = (hist.astype(np.float32) / n_sel).astype(np.float32)
    return x, density


# revision 24
# speedup vs baseline: 1.8428x; 1.8428x over previous
"""Masked 16-bin histogram (BinsCount) on 8 TRN2 NeuronCores.

Reference computation (see problem):
    m = broadcast(mask != 0); in_range = (x >= e0) & (x <= e16)
    idx = clip(searchsorted(edges, x, 'right') - 1, 0, 15)
    hist = bincount(idx, weights=m & in_range); density = hist / sum(m)
    returns (x, density)

Device algorithm: for j in 0..16: S_j = #{x' >= e_j}, plus T = #{x' > e16},
where x' = fp16(x) + m'' and m'' = 0 where mask kept, -inf where dropped
(so dropped elements become -inf and vanish from every count).  Exactly:
    hist[k] = S_k - S_{k+1}  (k < 15),   hist[15] = S_15 - T
n_selected = 32 * count(mask != 0) (host, exact).

Engine split per [128, 2048] tile (cost-model rates):
    gpsimd SWDGE DMA : cast-load x f32->fp16, then SBUF->SBUF accum-add of
                       the resident fp16 mask tile (masking costs no
                       compute-engine time)          ~3.2 us
    VectorE          : 14 edges, fused is_ge+accum fp16 (~0.61 us each)
    ScalarE          : 4 edges via Sign activation + accum (~2.1 us each)

Sharding: data-parallel along S (dim 2): core i takes x[:, :, i*256:(i+1)*256, :]
(64 MiB) + matching mask rows; the (B,H)-broadcast of the mask is handled
on-device by tile layout alignment.

fp16 note: x is rounded to fp16 before binning; only elements within half
an fp16 ulp of a bin edge can change bins (~0.1% of a bin's mass), far
inside the 2e-2 tolerance.  Counts themselves are exact integers in f32
accumulators.
"""

import os
import sys

import numpy as np

for _p in ("/opt/trn_rl_repo",):
    if _p not in sys.path:
        sys.path.insert(0, _p)

B, H, S, C = 2, 16, 2048, 2048
NCORES = 8
S_SH = S // NCORES              # 256 rows of S per core
TILES = (B * H * S_SH) // 128   # 64 tiles of [128, C] per core
NEDGE = 18                      # S_0..S_16 (>=), T (> e16)

# Edge -> engine assignment (indices into the 18 logical counters).
# Measured-on-HW optimum: fused compare+accum runs ~1.69us/tile on VectorE,
# Sign+accum ~2.11us/tile on ScalarE -> 10/8 balances both at ~16.9us/tile.
# (SWDGE accum-add offload of edges was tried and measured ~6us/transfer on
# silicon -> strictly worse; keep DMA_EDGES empty.)
DVE_EDGES = tuple(range(0, 10))      # fused ts+accum on VectorE
DMA_EDGES = ()                       # disabled (see above)
ACT_EDGES = tuple(range(10, 18))     # Sign+accum on ScalarE

LAST_EXEC_NS = None
LAST_RESULTS = None


def _build(edges17):
    """Raw-bass SPMD graph; edges17: 17 python floats (sorted)."""
    import concourse.bass as bass
    import concourse.mybir as mybir

    f32 = mybir.dt.float32
    f16 = mybir.dt.float16
    Alu = mybir.AluOpType
    ACT = mybir.ActivationFunctionType

    MBUF = 8                 # xm ring (gps DMA -> DVE/ACT readers)
    IRING = 2                # indicator ring depth (DVE -> SWDGE accum)
    ND = len(DMA_EDGES)
    RS = S_SH // 128         # mask row-tiles (2)

    def cmp_of(e):
        return Alu.is_ge if e < 17 else Alu.is_gt

    def val_of(e):
        return float(edges17[min(e, 16)])

    nc = bass.Bass()
    x_ext = nc.declare_dram_parameter("x", [B * H, RS, 128, C], f32,
                                      isOutput=False)
    mp_ext = nc.declare_dram_parameter("mp", [RS, 128, C], f16,
                                       isOutput=False)
    bias_ext = nc.declare_dram_parameter("bias", [128, NEDGE], f32,
                                         isOutput=False)
    out_ext = nc.declare_dram_parameter("out", [128, NEDGE], f32, isOutput=True)

    with (
        nc.sbuf_tensor([128, MBUF * C], f16) as xmbuf,
        nc.sbuf_tensor([128, RS * C], f16) as mbuf,
        nc.sbuf_tensor([128, NEDGE], f32) as biasbuf,
        nc.sbuf_tensor([128, C], f16) as scr_v,
        nc.sbuf_tensor([128, C], f16) as scr_a,
        nc.sbuf_tensor([128, max(1, IRING * ND) * C], f16) as indbuf,
        nc.sbuf_tensor([128, max(1, ND) * C], f16) as accd,
        nc.sbuf_tensor([128, NEDGE * TILES], f32) as acc,
        nc.sbuf_tensor([128, NEDGE], f32) as accf,
        nc.semaphore("dma_b") as dma_b,     # sync loads: bias, mask
        nc.semaphore("xc_sem") as xc_sem,   # gps cast-load completions
        nc.semaphore("xm_sem") as xm_sem,   # gps mask-add completions
        nc.semaphore("ind_sem") as ind_sem,  # DVE indicator (t,j) -> 6t+j+1
        nc.semaphore("dma_acc") as dma_acc,  # SWDGE accum completions (16 ea)
        nc.semaphore("zinit") as zinit,     # accd zero-init done
        nc.semaphore("vdone") as vdone,     # DVE finished tile t -> t+1
        nc.semaphore("adone") as adone,     # ACT finished tile t -> t+1
        nc.semaphore("done_sem") as done_sem,
        nc.Block() as block,
    ):
        def slot_ap(e, t):
            return acc[:, e * TILES + t: e * TILES + t + 1]

        @block.sync
        def _(sync):
            sync.dma_start(out=biasbuf[:], in_=bias_ext[:]).then_inc(dma_b, 16)
            for r in range(RS):
                sync.dma_start(out=mbuf[:, r * C:(r + 1) * C],
                               in_=mp_ext[r]).then_inc(dma_b, 16)
            sync.wait_ge(done_sem, 1)
            sync.dma_start(out=out_ext[:], in_=accf[:]).then_inc(dma_b, 16)

        @block.gpsimd
        def _(gpsimd):
            gpsimd.wait_ge(dma_b, 16 * (1 + RS))
            if ND:
                gpsimd.wait_ge(zinit, 1)
            for t in range(TILES):
                bh, r = divmod(t, RS)
                m = t % MBUF
                mslot = xmbuf[:, m * C:(m + 1) * C]
                if t >= MBUF:
                    # xm slot reused: DVE/ACT readers of tile t-MBUF done
                    gpsimd.wait_ge(vdone, t - MBUF + 1)
                    gpsimd.wait_ge(adone, t - MBUF + 1)
                # cast-load f32 -> fp16
                gpsimd.dma_start(out=mslot,
                                 in_=x_ext[bh, r]).then_inc(xc_sem, 16)
                gpsimd.wait_ge(xc_sem, 16 * (t + 1))
                # fp16 masking: += (0 | -inf)
                gpsimd.dma_start(out=mslot, in_=mbuf[:, r * C:(r + 1) * C],
                                 accum_op=Alu.add).then_inc(xm_sem, 16)
                # SWDGE-accumulated edges: accd[j] += ind16(t, j).
                # Full serialization of accum transfers (wait for ALL prior
                # accum completions before enqueueing) keeps RMW race-free
                # even if descriptors fan out across queues.
                base = (t % IRING) * ND
                for j in range(ND):
                    gpsimd.wait_ge(ind_sem, ND * t + j + 1)
                    if t > 0 or j > 0:
                        gpsimd.wait_ge(dma_acc, 16 * (ND * t + j))
                    gpsimd.dma_start(
                        out=accd[:, j * C:(j + 1) * C],
                        in_=indbuf[:, (base + j) * C:(base + j + 1) * C],
                        accum_op=Alu.add).then_inc(dma_acc, 16)

        @block.vector
        def _(vector):
            # zero-init the SWDGE accumulators before any accum-add lands
            if ND:
                nc.vector.memset(accd[:], 0.0).then_inc(zinit, 1)
            for t in range(TILES):
                m = t % MBUF
                xs = xmbuf[:, m * C:(m + 1) * C]
                vector.wait_ge(xm_sem, 16 * (t + 1))
                if ND and t >= IRING:
                    # indicator ring slot reuse: all accums of tile t-IRING
                    # must have completed
                    vector.wait_ge(dma_acc, 16 * ND * (t - IRING + 1))
                base = (t % IRING) * ND
                for j, e in enumerate(DMA_EDGES):
                    nc.vector.tensor_scalar(
                        out=indbuf[:, (base + j) * C:(base + j + 1) * C],
                        in0=xs, scalar1=val_of(e), scalar2=None,
                        op0=cmp_of(e)).then_inc(ind_sem, 1)
                last = None
                for e in DVE_EDGES:
                    last = nc.vector.tensor_scalar(
                        out=scr_v[:], in0=xs,
                        scalar1=val_of(e), scalar2=None,
                        op0=cmp_of(e), op1=Alu.add, accum_out=slot_ap(e, t))
                last.then_inc(vdone, 1)
            vector.wait_ge(adone, TILES)
            if ND:
                vector.wait_ge(dma_acc, 16 * ND * TILES)
            for e in range(NEDGE):
                if e in DMA_EDGES:
                    j = DMA_EDGES.index(e)
                    ins = nc.vector.tensor_reduce(
                        out=accf[:, e:e + 1],
                        in_=accd[:, j * C:(j + 1) * C],
                        axis=mybir.AxisListType.X, op=Alu.add)
                else:
                    ins = nc.vector.tensor_reduce(
                        out=accf[:, e:e + 1],
                        in_=acc[:, e * TILES:(e + 1) * TILES],
                        axis=mybir.AxisListType.X, op=Alu.add)
            ins.then_inc(done_sem, 1)

        @block.scalar
        def _(scalar):
            scalar.wait_ge(dma_b, 16)  # bias loaded
            for t in range(TILES):
                m = t % MBUF
                scalar.wait_ge(xm_sem, 16 * (t + 1))
                last = None
                for e in ACT_EDGES:
                    last = nc.scalar.activation(
                        out=scr_a[:], in_=xmbuf[:, m * C:(m + 1) * C],
                        func=ACT.Sign, bias=biasbuf[:, e:e + 1], scale=1.0,
                        accum_out=slot_ap(e, t))
                last.then_inc(adone, 1)

    return nc


_BUILD_CACHE = {}


def _get_nc(edges17):
    key = (tuple(edges17), DVE_EDGES, ACT_EDGES)
    if key not in _BUILD_CACHE:
        _BUILD_CACHE[key] = _build(edges17)
    return _BUILD_CACHE[key]


def kernel(x, mask, bins_edges):
    global LAST_EXEC_NS, LAST_RESULTS
    from concourse.bass_utils import run_bass_kernel_spmd

    x = np.asarray(x)
    mask = np.asarray(mask)
    edges = np.asarray(bins_edges, dtype=np.float32)
    assert x.shape == (B, H, S, C) and edges.shape == (17,)

    m2 = (mask.reshape(S, C) != 0)
    n_sel = np.float32(B * H) * np.float32(m2.sum(dtype=np.int64))
    mp16 = np.where(m2, np.float16(0.0), np.float16(-np.inf)).astype(np.float16)

    nc = _get_nc([float(v) for v in edges])

    bias = np.tile(-edges[np.minimum(np.arange(NEDGE), 16)].reshape(1, -1),
                   (128, 1)).astype(np.float32)
    in_maps = []
    for i in range(NCORES):
        xs = np.ascontiguousarray(
            x[:, :, i * S_SH:(i + 1) * S_SH, :]
        ).reshape(B * H, S_SH // 128, 128, C)
        mps = mp16[i * S_SH:(i + 1) * S_SH].reshape(S_SH // 128, 128, C)
        in_maps.append({"x": xs, "mp": np.ascontiguousarray(mps),
                        "bias": bias})

    trace = bool(int(os.environ.get("HIST_TRACE", "0")))
    res = run_bass_kernel_spmd(nc, in_maps, list(range(NCORES)), trace=trace)
    LAST_RESULTS = res
    LAST_EXEC_NS = res.exec_time_ns

    S_tot = np.zeros(NEDGE, dtype=np.float64)
    for i in range(NCORES):
        S_tot += res.results[i]["out"].astype(np.float64).sum(axis=0)

    n_elem_total = float(B * H * S * C)
    for e in ACT_EDGES:
        S_tot[e] = (S_tot[e] + n_elem_total) / 2.0

    hist = np.empty(16, dtype=np.float64)
    hist[:15] = S_tot[:15] - S_tot[1:16]
    hist[15] = S_tot[15] - S_tot[17]

    density = (hist.astype(np.float32) / n_sel).astype(np.float32)
    return x, density
